# revision 19
# baseline (speedup 1.0000x reference)
"""Bahdanau-attention kernel for Trainium2 (8 NeuronCores, data-parallel over batch).

Computation (per batch b):
    enc_proj = h_enc @ W1.T + b1          # (L, D)   -- the big matmul
    dec_proj = h_dec @ W2.T + b2          # (D,)
    h        = tanh(enc_proj + dec_proj)  # (L, D)
    scores   = h @ V (+ bv)               # (L,)  -- bv cancels in softmax, dropped
    attn     = softmax(scores)            # no-max softmax: |scores| small, exp is safe
    ctx      = attn @ enc_proj            # (D,)

Device layout: everything transposed ("T-space", e/d on partitions):
  - work unit is a BLOCK of 512 l-columns (4 per batch, 16 blocks/core).
  - startup: weights ship host-prepacked in [p, chunk, e] layout so the two
    weight DMAs are single big contiguous-per-partition transfers on the ACT
    HWDGE ring (a DMA's ~2.5us fixed cost made 17 small loads trickle in over
    ~80us and starve the PE).  Order: misc -> w1 -> w2+hdec.  dec_proj's
    matmul groups interleave into block 0's c-loop with a lag of 4 groups so
    the PE never head-of-line blocks on the later-arriving w2 pack.
  - enc_projT[e, l] accumulated in PSUM via lhsT=W1T tiles, rhs=h_encT tiles
  - h_encT via one cast-DMA (fp32->fp16 SWDGE) + one xbar DMA-transpose per
    block; transposes own the SP ring exclusively.
  - tanh fused with (b1+b2+dec_proj) bias on ACT; exp fused with Z-sum on ACT
  - scores: V-weighted partial sums over e-chunks on DVE (tensor_scalar mult
    + tensor_tensor add), then ONE ones-matmul per block contracts the 128
    partitions on PE (output replicated over rows -> free broadcast). The
    ones-matmul + exp of block i-1 are emitted right after block i's FIRST
    c-group, so block i-1's DVE ctx work overlaps block i's matmuls.
    (NOTE: scalar_tensor_tensor / tensor_tensor_reduce / memset / SWDGE
    gather-loads hang or crash the HW here -- stick to proven patterns.)
  - ctx via DVE tensor_tensor mult + tensor_reduce against evacuated
    enc_projT (fp16)
  - LAST block special-cased for tail latency: scores via replicated-V
    matmuls interleaved with the W1 groups (no DVE chain in the tail), and
    the ctx reduction splits even/odd chunks between ACT (activation
    accum_out) and DVE so the two engines pipeline.
  - divide by Z only at the very end; all 4 batch outputs staged in SBUF and
    shipped in a single end-of-kernel DMA.
"""

import numpy as np

B, L, D = 32, 2048, 1024
NCORES = 8
NB = B // NCORES  # batches per core
P = 128
NCH = D // P      # 8 chunks of the d/e dimension
BLK = 512         # l-columns per block (one PSUM bank of fp32)
NBK = L // BLK    # 4 blocks per batch
TB = BLK // P     # 4 xbar column-groups per block

_cache = {}


def _build():
    import concourse.bass as bass
    import concourse.tile as tile
    from concourse import bacc, mybir
    from concourse.bass import ts, ds
    from contextlib import ExitStack

    FP16 = mybir.dt.float16
    FP32 = mybir.dt.float32
    Alu = mybir.AluOpType
    Act = mybir.ActivationFunctionType
    X = mybir.AxisListType.X

    nc = bacc.Bacc("TRN2", name="bahdanau_attn")

    h_enc = nc.dram_tensor("h_enc", [NB, L, D], FP32, kind="ExternalInput")
    w1t = nc.dram_tensor("w1t", [P, NCH, D], FP16, kind="ExternalInput")       # [dpart, dchunk, e]
    w2p = nc.dram_tensor("w2p", [P, NCH, D + NB], FP16, kind="ExternalInput")  # w2t ++ hdec, same layout
    misc = nc.dram_tensor("misc", [P, 3 * NCH + P], FP32, kind="ExternalInput")  # b1|b12|v|ones
    out = nc.dram_tensor("ctx_out", [P, NB, NCH], FP32, kind="ExternalOutput")

    with tile.TileContext(nc) as tc, ExitStack() as ctx:
        wp = ctx.enter_context(tc.tile_pool(name="weights", bufs=1))
        l32 = ctx.enter_context(tc.tile_pool(name="ld32", bufs=2))
        ld = ctx.enter_context(tc.tile_pool(name="loads", bufs=4))
        tp = ctx.enter_context(tc.tile_pool(name="hT", bufs=4))
        ep = ctx.enter_context(tc.tile_pool(name="encproj", bufs=2))
        hp = ctx.enter_context(tc.tile_pool(name="htan", bufs=3))
        sa = ctx.enter_context(tc.tile_pool(name="sacc", bufs=2))
        xp = ctx.enter_context(tc.tile_pool(name="exps", bufs=2))
        sp = ctx.enter_context(tc.tile_pool(name="scratch", bufs=2))
        fin = ctx.enter_context(tc.tile_pool(name="final", bufs=2))
        psA = ctx.enter_context(tc.tile_pool(name="psA", bufs=5, space="PSUM"))
        psS = ctx.enter_context(tc.tile_pool(name="psS", bufs=2, space="PSUM"))
        psD = ctx.enter_context(tc.tile_pool(name="psD", bufs=1, space="PSUM"))

        # ---- prologue: 3 batched loads on the ACT HWDGE ring ----
        misc_sb = wp.tile([P, 3 * NCH + P], FP32)
        nc.scalar.dma_start(misc_sb, misc[:])
        w1_sb = wp.tile([P, NCH, D], FP16)
        nc.scalar.dma_start(w1_sb, w1t[:])
        w2_sb = wp.tile([P, NCH, D + NB], FP16)
        nc.scalar.dma_start(w2_sb, w2p[:])
        b1_sb = misc_sb[:, 0:NCH]
        b12_sb = misc_sb[:, NCH : 2 * NCH]
        v_sb = misc_sb[:, 2 * NCH : 3 * NCH]

        # all-ones lhsT for the cross-partition scores reduction (fp32->fp16)
        ones_sb = wp.tile([P, P], FP16)
        nc.vector.tensor_copy(ones_sb, misc_sb[:, 3 * NCH :])
        # V replicated to [P, NCH, P] fp16: last block's scores run on PE
        vrep = wp.tile([P, NCH, P], FP16)
        nc.vector.tensor_copy(vrep, v_sb[:, :, None].to_broadcast([P, NCH, P]))

        # bias_sb[:, c, b] = dec_proj[b, e] + b1[e] + b2[e]   (e = c*128 + p)
        bias_sb = wp.tile([P, NCH, NB], FP32)
        # per-batch output staging: out_all[p, b, c]
        out_all = wp.tile([P, NB, NCH], FP32)

        def emit_dec_group(c):
            psd = psD.tile([P, BLK], FP32, tag="dec")
            for d in range(NCH):
                nc.tensor.matmul(
                    psd[:, :NB],
                    lhsT=w2_sb[:, d, ts(c, P)],
                    rhs=w2_sb[:, d, D : D + NB],
                    start=(d == 0),
                    stop=(d == NCH - 1),
                )
            nc.vector.tensor_scalar(
                out=bias_sb[:, c, :], in0=psd[:, :NB],
                scalar1=b12_sb[:, c : c + 1], scalar2=None, op0=Alu.add,
            )

        # ---- software-pipelined main loop over 16 blocks ----
        NBLOCKS = NB * NBK
        LAST = NBLOCKS - 1
        DECLAG = 4
        batch_state = {}
        block_state = {}

        def front_begin(i):
            """load + transpose + first W1 c-group."""
            b, k = divmod(i, NBK)
            if k == 0:
                batch_state[b] = {
                    "enc": ep.tile([P, NCH, L], FP16, tag="enc", name="enc_sb"),
                    "exp": xp.tile([P, L], FP16, tag="exp", name="exp_rep"),
                    "zsl": fin.tile([P, NBK], FP32, tag="zsl", name="zsl"),
                    "ctx": fin.tile([P, NCH, NBK], FP32, tag="ctxsl", name="ctx_sl"),
                }
            lr = ds(k * BLK, BLK)
            nat = ld.tile([P, TB, D], FP16, tag="nat")
            if i < 3:
                # startup blocks: pure SWDGE cast (q0) -- the HWDGE ring is
                # still busy shipping weights
                nc.gpsimd.dma_start(
                    nat, h_enc[b, lr, :].rearrange("(t p) d -> p t d", p=P)
                )
            else:
                # steady state: split the load across both DMA queues (each
                # caps at ~180 GB/s; a full 2MiB fp32 read on one queue paces
                # blocks slower than the PE). First half SWDGE cast on q0,
                # second half fp32 on the now-idle ACT ring + DVE cast.
                nc.gpsimd.dma_start(
                    nat[:, 0 : TB // 2, :],
                    h_enc[b, ds(k * BLK, BLK // 2), :].rearrange("(t p) d -> p t d", p=P),
                )
                nat32h = l32.tile([P, TB // 2, D], FP32, tag="n32")
                nc.scalar.dma_start(
                    nat32h,
                    h_enc[b, ds(k * BLK + BLK // 2, BLK // 2), :].rearrange("(t p) d -> p t d", p=P),
                )
                nc.vector.tensor_copy(nat[:, TB // 2 :, :], nat32h)
            hT = tp.tile([P, TB, NCH, P], FP16, tag="hT")
            nc.sync.dma_start(hT, nat.rearrange("p t d -> p (t d)"), transpose=True)
            block_state[i] = {"hT": hT, "ps_sc": None, "sacc": None, "htans": []}
            if i == 0:
                # block 0: emit W1 MM groups ahead of the tanh/sacc parts so
                # dec_proj's groups (gated on the later-arriving w2 pack) can
                # interleave without ever head-of-line blocking the W1 MMs
                pss = [emit_mms(0, c) for c in range(DECLAG)]
                for c in range(NCH):
                    emit_dec_group(c)
                    emit_act(0, c, pss[c])
                    if c + DECLAG < NCH:
                        pss.append(emit_mms(0, c + DECLAG))
            else:
                emit_c_group(i, 0)

        def emit_mms(i, c):
            """one e-chunk's 8 W1 matmuls."""
            bst = block_state[i]
            ps = psA.tile([P, BLK], FP32, tag="mm")
            for d in range(NCH):
                nc.tensor.matmul(
                    ps,
                    lhsT=w1_sb[:, d, ts(c, P)],
                    rhs=bst["hT"][:, :, d, :],
                    start=(d == 0),
                    stop=(d == NCH - 1),
                )
            return ps

        def emit_c_group(i, c):
            """one e-chunk: 8 W1 matmuls + tanh + evac + scores partial."""
            ps = emit_mms(i, c)
            emit_act(i, c, ps)

        def emit_act(i, c, ps):
            b, k = divmod(i, NBK)
            st = batch_state[b]
            bst = block_state[i]
            lr = ds(k * BLK, BLK)
            htan = hp.tile([P, BLK], FP16, tag="htan")
            nc.scalar.activation(htan, ps, Act.Tanh, bias=bias_sb[:, c, b : b + 1])
            nc.scalar.activation(st["enc"][:, c, lr], ps, Act.Identity, bias=b1_sb[:, c : c + 1])

            if i == LAST:
                # tail path: scores on PE with replicated V, lagged one group
                bst["htans"].append(htan)
                if c > 0:
                    if bst["ps_sc"] is None:
                        bst["ps_sc"] = psS.tile([P, BLK], FP32, tag="sc", name="ps_sc")
                    nc.tensor.matmul(
                        bst["ps_sc"], lhsT=vrep[:, c - 1, :], rhs=bst["htans"][c - 1],
                        start=(c - 1 == 0), stop=False,
                    )
                if c == NCH - 1:
                    nc.tensor.matmul(
                        bst["ps_sc"], lhsT=vrep[:, c, :], rhs=bst["htans"][c],
                        start=False, stop=True,
                    )
                return
            # V-weighted partial sums for scores on DVE:
            #   sacc[p, l] = sum_c v[p, c] * htan_c[p, l]
            with nc.allow_low_precision("fp16 partials; |sacc| < 1"):
                nxt = sa.tile([P, BLK], FP16, tag="sacc")
                if c == 0:
                    nc.vector.tensor_scalar(
                        out=nxt, in0=htan,
                        scalar1=v_sb[:, 0:1], scalar2=None, op0=Alu.mult,
                    )
                else:
                    prod = sa.tile([P, BLK], FP16, tag="sprod")
                    nc.vector.tensor_scalar(
                        out=prod, in0=htan,
                        scalar1=v_sb[:, c : c + 1], scalar2=None, op0=Alu.mult,
                    )
                    nc.vector.tensor_tensor(nxt, prod, bst["sacc"], Alu.add)
                bst["sacc"] = nxt

        def front_rest(i):
            if i == 0:
                return  # block 0 fully emitted in front_begin
            for c in range(1, NCH):
                emit_c_group(i, c)

        def tail_scores(i):
            """ones-matmul + exp+Z of block i (emitted early in block i+1)."""
            b, k = divmod(i, NBK)
            st = batch_state[b]
            bst = block_state[i]
            lr = ds(k * BLK, BLK)
            if i != LAST:
                ps_sc = psS.tile([P, BLK], FP32, tag="sc")
                nc.tensor.matmul(ps_sc, lhsT=ones_sb, rhs=bst["sacc"], start=True, stop=True)
                bst["ps_sc"] = ps_sc
            nc.scalar.activation(
                st["exp"][:, lr], bst["ps_sc"], Act.Exp,
                accum_out=st["zsl"][:, k : k + 1],
            )

        def tail_ctx(i):
            """ctx partials of block i; batch finalize on its last block."""
            b, k = divmod(i, NBK)
            st = batch_state[b]
            lr = ds(k * BLK, BLK)
            del block_state[i]
            with nc.allow_low_precision("fp16 block partials; |ctx_unnorm|<~1e3"):
                for c in range(NCH):
                    scratch = sp.tile([P, BLK], FP16, tag="ttr")
                    nc.vector.tensor_tensor(
                        scratch, st["enc"][:, c, lr], st["exp"][:, lr], Alu.mult
                    )
                    if i == LAST and c % 2 == 0:
                        # reduce on ACT so it pipelines with DVE's multiplies
                        sink = sp.tile([P, BLK], FP16, tag="ttr2", name="sink")
                        nc.scalar.activation(
                            sink, scratch,
                            Act.Identity, accum_out=st["ctx"][:, c, k : k + 1],
                        )
                    else:
                        nc.vector.tensor_reduce(
                            st["ctx"][:, c, k : k + 1], scratch, axis=X, op=Alu.add
                        )

            if k == NBK - 1:
                # finalize: ctx = ctx_unnorm / Z
                zsum = fin.tile([P, 1], FP32, tag="zsum")
                nc.vector.tensor_reduce(zsum, st["zsl"], axis=X, op=Alu.add)
                recip = fin.tile([P, 1], FP32, tag="recip")
                nc.vector.reciprocal(recip, zsum)
                ctxr = fin.tile([P, NCH], FP32, tag="ctxr")
                nc.vector.tensor_reduce(ctxr, st["ctx"], axis=X, op=Alu.add)
                nc.vector.tensor_scalar(
                    out=out_all[:, b, :], in0=ctxr, scalar1=recip,
                    scalar2=None, op0=Alu.mult,
                )
                del batch_state[b]

        for i in range(NBLOCKS + 1):
            if i < NBLOCKS:
                front_begin(i)
            if i >= 1:
                tail_scores(i - 1)
            if i < NBLOCKS:
                front_rest(i)
            if i >= 1:
                tail_ctx(i - 1)

        # single end-of-kernel output DMA (16 KiB)
        nc.sync.dma_start(out[:], out_all)

    nc.finalize()
    return nc


def _prep_shared(W1, b1, W2, b2, V):
    f16 = np.float16
    # [dpart, dchunk, e] prepacked so the device DMA is contiguous/partition
    w1t = np.ascontiguousarray(W1.T.reshape(NCH, P, D).transpose(1, 0, 2).astype(f16))
    w2t = W2.T.reshape(NCH, P, D).transpose(1, 0, 2).astype(f16)
    b1t = b1.reshape(NCH, P).T.astype(np.float32)
    b12t = (b1 + b2).reshape(NCH, P).T.astype(np.float32)
    vt = V.reshape(NCH, P).T.astype(np.float32)
    misc = np.ascontiguousarray(
        np.concatenate([b1t, b12t, vt, np.ones((P, P), np.float32)], axis=1)
    )
    return w1t, w2t, misc


def kernel(h_enc, h_dec, W1, b1, W2, b2, V, bv):
    from concourse.bass_utils import run_bass_kernel_spmd

    h_enc = np.asarray(h_enc, dtype=np.float32)
    h_dec = np.asarray(h_dec, dtype=np.float32)
    W1 = np.asarray(W1, dtype=np.float32)
    b1 = np.asarray(b1, dtype=np.float32)
    W2 = np.asarray(W2, dtype=np.float32)
    b2 = np.asarray(b2, dtype=np.float32)
    V = np.asarray(V, dtype=np.float32)

    if "nc" not in _cache:
        _cache["nc"] = _build()
    nc = _cache["nc"]

    w1t, w2t, misc = _prep_shared(W1, b1, W2, b2, V)

    in_maps = []
    for core in range(NCORES):
        sl = slice(core * NB, (core + 1) * NB)
        hdect = h_dec[sl].T.reshape(NCH, P, NB).transpose(1, 0, 2).astype(np.float16)
        w2pk = np.ascontiguousarray(np.concatenate([w2t, hdect], axis=2))
        in_maps.append(
            {
                "h_enc": np.ascontiguousarray(h_enc[sl]),
                "w1t": w1t,
                "w2p": w2pk,
                "misc": misc,
            }
        )

    res = run_bass_kernel_spmd(nc, in_maps, core_ids=list(range(NCORES)))
    _cache["last_results"] = res
    outs = []
    for core in range(NCORES):
        o = res.results[core]["ctx_out"]  # [P, NB, NCH]
        outs.append(o.transpose(1, 2, 0).reshape(NB, D))  # e = c*128 + p
    return np.concatenate(outs, axis=0).astype(np.float32)


# revision 20
# speedup vs baseline: 1.0513x; 1.0513x over previous
"""Bahdanau-attention kernel for Trainium2 (8 NeuronCores, data-parallel over batch).

Computation (per batch b):
    enc_proj = h_enc @ W1.T + b1          # (L, D)   -- the big matmul
    dec_proj = h_dec @ W2.T + b2          # (D,)
    h        = tanh(enc_proj + dec_proj)  # (L, D)
    scores   = h @ V (+ bv)               # (L,)  -- bv cancels in softmax, dropped
    attn     = softmax(scores)            # no-max softmax: |scores| small, exp is safe
    ctx      = attn @ enc_proj            # (D,)

Device layout: everything transposed ("T-space", e/d on partitions):
  - work unit is a BLOCK of 512 l-columns (4 per batch, 16 blocks/core).
  - startup: weights ship host-prepacked in [p, chunk, e] layout so the two
    weight DMAs are single big contiguous-per-partition transfers on the ACT
    HWDGE ring (a DMA's ~2.5us fixed cost made 17 small loads trickle in over
    ~80us and starve the PE).  Order: misc -> w1 -> w2+hdec.  dec_proj's
    matmul groups interleave into block 0's c-loop with a lag of 4 groups so
    the PE never head-of-line blocks on the later-arriving w2 pack.
  - enc_projT[e, l] accumulated in PSUM via lhsT=W1T tiles, rhs=h_encT tiles
  - h_encT via one cast-DMA (fp32->fp16 SWDGE) + one xbar DMA-transpose per
    block; transposes own the SP ring exclusively.
  - tanh fused with (b1+b2+dec_proj) bias on ACT; exp fused with Z-sum on ACT
  - scores: V-weighted partial sums over e-chunks on DVE (tensor_scalar mult
    + tensor_tensor add), then ONE ones-matmul per block contracts the 128
    partitions on PE (output replicated over rows -> free broadcast). The
    ones-matmul + exp of block i-1 are emitted right after block i's FIRST
    c-group, so block i-1's DVE ctx work overlaps block i's matmuls.
    (NOTE: scalar_tensor_tensor / tensor_tensor_reduce / memset / SWDGE
    gather-loads hang or crash the HW here -- stick to proven patterns.)
  - ctx via DVE tensor_tensor mult + tensor_reduce against evacuated
    enc_projT (fp16)
  - LAST block special-cased for tail latency: scores via replicated-V
    matmuls interleaved with the W1 groups (no DVE chain in the tail), and
    the ctx reduction splits even/odd chunks between ACT (activation
    accum_out) and DVE so the two engines pipeline.
  - divide by Z only at the very end; all 4 batch outputs staged in SBUF and
    shipped in a single end-of-kernel DMA.
"""

import numpy as np

B, L, D = 32, 2048, 1024
NCORES = 8
NB = B // NCORES  # batches per core
P = 128
NCH = D // P      # 8 chunks of the d/e dimension
BLK = 512         # l-columns per block (one PSUM bank of fp32)
NBK = L // BLK    # 4 blocks per batch
TB = BLK // P     # 4 xbar column-groups per block

_cache = {}


def _build():
    import concourse.bass as bass
    import concourse.tile as tile
    from concourse import bacc, mybir
    from concourse.bass import ts, ds
    from contextlib import ExitStack

    FP16 = mybir.dt.float16
    FP32 = mybir.dt.float32
    Alu = mybir.AluOpType
    Act = mybir.ActivationFunctionType
    X = mybir.AxisListType.X

    nc = bacc.Bacc("TRN2", name="bahdanau_attn")

    h_enc = nc.dram_tensor("h_enc", [NB, L, D], FP32, kind="ExternalInput")
    w1t = nc.dram_tensor("w1t", [P, NCH, D], FP16, kind="ExternalInput")       # [dpart, dchunk, e]
    w2p = nc.dram_tensor("w2p", [P, NCH, D + NB], FP16, kind="ExternalInput")  # w2t ++ hdec, same layout
    misc = nc.dram_tensor("misc", [P, 3 * NCH + P], FP32, kind="ExternalInput")  # b1|b12|v|ones
    out = nc.dram_tensor("ctx_out", [P, NB, NCH], FP32, kind="ExternalOutput")

    with tile.TileContext(nc) as tc, ExitStack() as ctx:
        wp = ctx.enter_context(tc.tile_pool(name="weights", bufs=1))
        ld = ctx.enter_context(tc.tile_pool(name="loads", bufs=4))
        tp = ctx.enter_context(tc.tile_pool(name="hT", bufs=4))
        ep = ctx.enter_context(tc.tile_pool(name="encproj", bufs=2))
        hp = ctx.enter_context(tc.tile_pool(name="htan", bufs=3))
        sa = ctx.enter_context(tc.tile_pool(name="sacc", bufs=2))
        xp = ctx.enter_context(tc.tile_pool(name="exps", bufs=2))
        sp = ctx.enter_context(tc.tile_pool(name="scratch", bufs=2))
        fin = ctx.enter_context(tc.tile_pool(name="final", bufs=2))
        psA = ctx.enter_context(tc.tile_pool(name="psA", bufs=5, space="PSUM"))
        psS = ctx.enter_context(tc.tile_pool(name="psS", bufs=2, space="PSUM"))
        psD = ctx.enter_context(tc.tile_pool(name="psD", bufs=1, space="PSUM"))

        # ---- prologue: 3 batched loads on the ACT HWDGE ring ----
        misc_sb = wp.tile([P, 3 * NCH + P], FP32)
        nc.scalar.dma_start(misc_sb, misc[:])
        w1_sb = wp.tile([P, NCH, D], FP16)
        nc.scalar.dma_start(w1_sb, w1t[:])
        w2_sb = wp.tile([P, NCH, D + NB], FP16)
        nc.scalar.dma_start(w2_sb, w2p[:])
        b1_sb = misc_sb[:, 0:NCH]
        b12_sb = misc_sb[:, NCH : 2 * NCH]
        v_sb = misc_sb[:, 2 * NCH : 3 * NCH]

        # all-ones lhsT for the cross-partition scores reduction (fp32->fp16)
        ones_sb = wp.tile([P, P], FP16)
        nc.vector.tensor_copy(ones_sb, misc_sb[:, 3 * NCH :])
        # V replicated to [P, NCH, P] fp16: last block's scores run on PE
        vrep = wp.tile([P, NCH, P], FP16)
        nc.vector.tensor_copy(vrep, v_sb[:, :, None].to_broadcast([P, NCH, P]))

        # bias_sb[:, c, b] = dec_proj[b, e] + b1[e] + b2[e]   (e = c*128 + p)
        bias_sb = wp.tile([P, NCH, NB], FP32)
        # per-batch output staging: out_all[p, b, c]
        out_all = wp.tile([P, NB, NCH], FP32)

        def emit_dec_group(c):
            psd = psD.tile([P, BLK], FP32, tag="dec")
            for d in range(NCH):
                nc.tensor.matmul(
                    psd[:, :NB],
                    lhsT=w2_sb[:, d, ts(c, P)],
                    rhs=w2_sb[:, d, D : D + NB],
                    start=(d == 0),
                    stop=(d == NCH - 1),
                )
            nc.vector.tensor_scalar(
                out=bias_sb[:, c, :], in0=psd[:, :NB],
                scalar1=b12_sb[:, c : c + 1], scalar2=None, op0=Alu.add,
            )

        # ---- software-pipelined main loop over 16 blocks ----
        NBLOCKS = NB * NBK
        LAST = NBLOCKS - 1
        DECLAG = 4
        batch_state = {}
        block_state = {}

        def front_begin(i):
            """load + transpose + first W1 c-group."""
            b, k = divmod(i, NBK)
            if k == 0:
                batch_state[b] = {
                    "enc": ep.tile([P, NCH, L], FP16, tag="enc", name="enc_sb"),
                    "exp": xp.tile([P, L], FP16, tag="exp", name="exp_rep"),
                    "zsl": fin.tile([P, NBK], FP32, tag="zsl", name="zsl"),
                    "ctx": fin.tile([P, NCH, NBK], FP32, tag="ctxsl", name="ctx_sl"),
                }
            lr = ds(k * BLK, BLK)
            nat = ld.tile([P, TB, D], FP16, tag="nat")
            nc.gpsimd.dma_start(
                nat, h_enc[b, lr, :].rearrange("(t p) d -> p t d", p=P)
            )
            hT = tp.tile([P, TB, NCH, P], FP16, tag="hT")
            nc.sync.dma_start(hT, nat.rearrange("p t d -> p (t d)"), transpose=True)
            block_state[i] = {"hT": hT, "ps_sc": None, "sacc": None, "htans": []}
            if i == 0:
                # block 0: emit W1 MM groups ahead of the tanh/sacc parts so
                # dec_proj's groups (gated on the later-arriving w2 pack) can
                # interleave without ever head-of-line blocking the W1 MMs
                pss = [emit_mms(0, c) for c in range(DECLAG)]
                for c in range(NCH):
                    emit_dec_group(c)
                    emit_act(0, c, pss[c])
                    if c + DECLAG < NCH:
                        pss.append(emit_mms(0, c + DECLAG))
            else:
                emit_c_group(i, 0)

        def emit_mms(i, c):
            """one e-chunk's 8 W1 matmuls."""
            bst = block_state[i]
            ps = psA.tile([P, BLK], FP32, tag="mm")
            for d in range(NCH):
                nc.tensor.matmul(
                    ps,
                    lhsT=w1_sb[:, d, ts(c, P)],
                    rhs=bst["hT"][:, :, d, :],
                    start=(d == 0),
                    stop=(d == NCH - 1),
                )
            return ps

        def emit_c_group(i, c):
            """one e-chunk: 8 W1 matmuls + tanh + evac + scores partial."""
            ps = emit_mms(i, c)
            emit_act(i, c, ps)

        def emit_act(i, c, ps):
            b, k = divmod(i, NBK)
            st = batch_state[b]
            bst = block_state[i]
            lr = ds(k * BLK, BLK)
            htan = hp.tile([P, BLK], FP16, tag="htan")
            nc.scalar.activation(htan, ps, Act.Tanh, bias=bias_sb[:, c, b : b + 1])
            nc.scalar.activation(st["enc"][:, c, lr], ps, Act.Identity, bias=b1_sb[:, c : c + 1])

            if i == LAST:
                # tail path: scores on PE with replicated V, lagged one group
                bst["htans"].append(htan)
                if c > 0:
                    if bst["ps_sc"] is None:
                        bst["ps_sc"] = psS.tile([P, BLK], FP32, tag="sc", name="ps_sc")
                    nc.tensor.matmul(
                        bst["ps_sc"], lhsT=vrep[:, c - 1, :], rhs=bst["htans"][c - 1],
                        start=(c - 1 == 0), stop=False,
                    )
                if c == NCH - 1:
                    nc.tensor.matmul(
                        bst["ps_sc"], lhsT=vrep[:, c, :], rhs=bst["htans"][c],
                        start=False, stop=True,
                    )
                return
            # V-weighted partial sums for scores on DVE:
            #   sacc[p, l] = sum_c v[p, c] * htan_c[p, l]
            with nc.allow_low_precision("fp16 partials; |sacc| < 1"):
                nxt = sa.tile([P, BLK], FP16, tag="sacc")
                if c == 0:
                    nc.vector.tensor_scalar(
                        out=nxt, in0=htan,
                        scalar1=v_sb[:, 0:1], scalar2=None, op0=Alu.mult,
                    )
                else:
                    prod = sa.tile([P, BLK], FP16, tag="sprod")
                    nc.vector.tensor_scalar(
                        out=prod, in0=htan,
                        scalar1=v_sb[:, c : c + 1], scalar2=None, op0=Alu.mult,
                    )
                    nc.vector.tensor_tensor(nxt, prod, bst["sacc"], Alu.add)
                bst["sacc"] = nxt

        def front_rest(i):
            if i == 0:
                return  # block 0 fully emitted in front_begin
            for c in range(1, NCH):
                emit_c_group(i, c)

        def tail_scores(i):
            """ones-matmul + exp+Z of block i (emitted early in block i+1)."""
            b, k = divmod(i, NBK)
            st = batch_state[b]
            bst = block_state[i]
            lr = ds(k * BLK, BLK)
            if i != LAST:
                ps_sc = psS.tile([P, BLK], FP32, tag="sc")
                nc.tensor.matmul(ps_sc, lhsT=ones_sb, rhs=bst["sacc"], start=True, stop=True)
                bst["ps_sc"] = ps_sc
            nc.scalar.activation(
                st["exp"][:, lr], bst["ps_sc"], Act.Exp,
                accum_out=st["zsl"][:, k : k + 1],
            )

        def tail_ctx(i):
            """ctx partials of block i; batch finalize on its last block."""
            b, k = divmod(i, NBK)
            st = batch_state[b]
            lr = ds(k * BLK, BLK)
            del block_state[i]
            with nc.allow_low_precision("fp16 block partials; |ctx_unnorm|<~1e3"):
                for c in range(NCH):
                    scratch = sp.tile([P, BLK], FP16, tag="ttr")
                    nc.vector.tensor_tensor(
                        scratch, st["enc"][:, c, lr], st["exp"][:, lr], Alu.mult
                    )
                    if i == LAST and c % 2 == 0:
                        # reduce on ACT so it pipelines with DVE's multiplies
                        sink = sp.tile([P, BLK], FP16, tag="ttr2", name="sink")
                        nc.scalar.activation(
                            sink, scratch,
                            Act.Identity, accum_out=st["ctx"][:, c, k : k + 1],
                        )
                    else:
                        nc.vector.tensor_reduce(
                            st["ctx"][:, c, k : k + 1], scratch, axis=X, op=Alu.add
                        )

            if k == NBK - 1:
                # finalize: ctx = ctx_unnorm / Z
                zsum = fin.tile([P, 1], FP32, tag="zsum")
                nc.vector.tensor_reduce(zsum, st["zsl"], axis=X, op=Alu.add)
                recip = fin.tile([P, 1], FP32, tag="recip")
                nc.vector.reciprocal(recip, zsum)
                ctxr = fin.tile([P, NCH], FP32, tag="ctxr")
                nc.vector.tensor_reduce(ctxr, st["ctx"], axis=X, op=Alu.add)
                nc.vector.tensor_scalar(
                    out=out_all[:, b, :], in0=ctxr, scalar1=recip,
                    scalar2=None, op0=Alu.mult,
                )
                del batch_state[b]

        for i in range(NBLOCKS + 1):
            if i < NBLOCKS:
                front_begin(i)
            if i >= 1:
                tail_scores(i - 1)
            if i < NBLOCKS:
                front_rest(i)
            if i >= 1:
                tail_ctx(i - 1)

        # single end-of-kernel output DMA (16 KiB)
        nc.sync.dma_start(out[:], out_all)

    nc.finalize()
    return nc


def _prep_shared(W1, b1, W2, b2, V):
    f16 = np.float16
    # [dpart, dchunk, e] prepacked so the device DMA is contiguous/partition
    w1t = np.ascontiguousarray(W1.T.reshape(NCH, P, D).transpose(1, 0, 2).astype(f16))
    w2t = W2.T.reshape(NCH, P, D).transpose(1, 0, 2).astype(f16)
    b1t = b1.reshape(NCH, P).T.astype(np.float32)
    b12t = (b1 + b2).reshape(NCH, P).T.astype(np.float32)
    vt = V.reshape(NCH, P).T.astype(np.float32)
    misc = np.ascontiguousarray(
        np.concatenate([b1t, b12t, vt, np.ones((P, P), np.float32)], axis=1)
    )
    return w1t, w2t, misc


def kernel(h_enc, h_dec, W1, b1, W2, b2, V, bv):
    from concourse.bass_utils import run_bass_kernel_spmd

    h_enc = np.asarray(h_enc, dtype=np.float32)
    h_dec = np.asarray(h_dec, dtype=np.float32)
    W1 = np.asarray(W1, dtype=np.float32)
    b1 = np.asarray(b1, dtype=np.float32)
    W2 = np.asarray(W2, dtype=np.float32)
    b2 = np.asarray(b2, dtype=np.float32)
    V = np.asarray(V, dtype=np.float32)

    if "nc" not in _cache:
        _cache["nc"] = _build()
    nc = _cache["nc"]

    w1t, w2t, misc = _prep_shared(W1, b1, W2, b2, V)

    in_maps = []
    for core in range(NCORES):
        sl = slice(core * NB, (core + 1) * NB)
        hdect = h_dec[sl].T.reshape(NCH, P, NB).transpose(1, 0, 2).astype(np.float16)
        w2pk = np.ascontiguousarray(np.concatenate([w2t, hdect], axis=2))
        in_maps.append(
            {
                "h_enc": np.ascontiguousarray(h_enc[sl]),
                "w1t": w1t,
                "w2p": w2pk,
                "misc": misc,
            }
        )

    res = run_bass_kernel_spmd(nc, in_maps, core_ids=list(range(NCORES)))
    _cache["last_results"] = res
    outs = []
    for core in range(NCORES):
        o = res.results[core]["ctx_out"]  # [P, NB, NCH]
        outs.append(o.transpose(1, 2, 0).reshape(NB, D))  # e = c*128 + p
    return np.concatenate(outs, axis=0).astype(np.float32)
